# revision 8
# baseline (speedup 1.0000x reference)
"""Cross-modal attention TRN2 kernel.

Problem: B=4, N=2048, IN_DIM=DIM=1024, HEADS=8, D_HEAD=128, scale=DIM**-0.5.
  q = x_a @ W_q.T ; k,v = split(x_b @ W_kv.T) ; per-head softmax(q k^T/32) v ;
  out = merge_heads @ W_out.T + b_out

Sharding over 8 cores: core c -> batch b=c//2, head-half hh=c%2 (4 heads,
512 of DIM).  W_q/W_kv column-sharded, W_out row-sharded (Megatron); each
core emits a partial output projection y_cT = (W_out[:, slice] @ O_half)
of shape [DIM, N]; host sums the two head-half partials per batch, adds
b_out, transposes back.

Device layout: everything transposed ([feature, token]) so all matmuls
contract over the partition dim.  Host feeds x^T and W^T (cheap numpy
prep); device does:
  phase 1: Q^T = WqT.T @ xaT, K^T likewise, V (natural [j, dv])
  phase 2: per (head, 1024-token block): dots^T = K_tile^T.T... i.e.
           s^T[j,i] = sum_d K^T[d,j] Q^T[d,i]; exp on ACT (no max
           subtraction -- |s*scale| < ~1 by construction of the problem
           scale); PV and a ones-row matmul (denominator) accumulate over
           j-tiles in PSUM; normalize with reciprocal broadcast.
  phase 3: y^T = WoT.T @ O^T, DMA PSUM->DRAM.
All matmuls run as float32r (full PE rate at N=512).
"""

import numpy as np

B, N, IN_DIM, DIM, HEADS = 4, 2048, 1024, 1024, 8
D_HEAD = DIM // HEADS          # 128
SCALE = DIM ** -0.5            # 1/32
NCORES = 8
HH = HEADS // 2                # 4 heads per core
DVC = HH * D_HEAD              # 512 dv per core
P = 128
KT = IN_DIM // P               # 8 contraction tiles
NJT = N // P                   # 16 j tiles
NIB = N // 512                 # 4 i-blocks of 512
IB2 = N // 1024                # 2 i-blocks of 1024

_TRACE = False
REPS = 1
LAST_EXEC_NS = None
LAST_RESULTS = None
_nc_cache = []


def _build_nc(reps=1):
    import concourse.tile as tile
    from concourse import bacc, mybir

    f32 = mybir.dt.float32
    f32r = mybir.dt.float32r
    Exp = mybir.ActivationFunctionType.Exp

    nc = bacc.Bacc("TRN2", debug=False, num_devices=NCORES)

    xaT = nc.dram_tensor("xaT", [IN_DIM, N], f32r, kind="ExternalInput").ap()
    xbT = nc.dram_tensor("xbT", [IN_DIM, N], f32r, kind="ExternalInput").ap()
    wqT = nc.dram_tensor("wqT", [IN_DIM, DVC], f32r, kind="ExternalInput").ap()
    wkT = nc.dram_tensor("wkT", [IN_DIM, DVC], f32r, kind="ExternalInput").ap()
    wvT = nc.dram_tensor("wvT", [IN_DIM, DVC], f32r, kind="ExternalInput").ap()
    woT = nc.dram_tensor("woT", [DVC, DIM], f32r, kind="ExternalInput").ap()
    ones_d = nc.dram_tensor("ones", [P, 1], f32r, kind="ExternalInput").ap()
    yT = nc.dram_tensor("yT", [DIM, N], f32, kind="ExternalOutput").ap()

    with tile.TileContext(nc) as tc:
      for _rep in range(reps):
        with tc.tile_pool(name="persist", bufs=1) as persist:
            qT_sb = persist.tile([P, HH, N], f32r)      # [d%128, head, i]
            kT_sb = persist.tile([P, HH, N], f32r)      # [d%128, head, j]
            v_sb = persist.tile([P, NJT, DVC], f32r)    # [j%128, jt, dv]
            oT_sb = persist.tile([P, HH, N], f32r)      # [dv%128, head, i]
            ones_sb = persist.tile([P, 1], f32r)
            nc.sync.dma_start(out=ones_sb, in_=ones_d)

            # ---------------- phase 1: projections ----------------
            BW = 256  # streaming block width (>=256 keeps f32r full rate)
            NB = N // BW
            with tc.tile_pool(name="wpool", bufs=1) as wpool, \
                 tc.tile_pool(name="xblk", bufs=3) as xblk, \
                 tc.tile_pool(name="psum1", bufs=4, space="PSUM") as psum1:
                wq_sb = wpool.tile([P, KT, DVC], f32r)
                wk_sb = wpool.tile([P, KT, DVC], f32r)
                wv_sb = wpool.tile([P, KT, DVC], f32r)
                nc.sync.dma_start(
                    out=wq_sb, in_=wqT.rearrange("(kt p) d -> p kt d", p=P))
                nc.sync.dma_start(
                    out=wk_sb, in_=wkT.rearrange("(kt p) d -> p kt d", p=P))
                nc.sync.dma_start(
                    out=wv_sb, in_=wvT.rearrange("(kt p) d -> p kt d", p=P))

                for ib in range(NB):
                    xa_blk = xblk.tile([P, KT, BW], f32r, tag="xblk")
                    nc.sync.dma_start(
                        out=xa_blk,
                        in_=xaT[:, ib * BW:(ib + 1) * BW]
                        .rearrange("(kt p) i -> p kt i", p=P))
                    for dt in range(HH):
                        ps = psum1.tile([P, BW], f32, tag="ps1")
                        for kt in range(KT):
                            nc.tensor.matmul(
                                ps,
                                wq_sb[:, kt, dt * P:(dt + 1) * P],
                                xa_blk[:, kt, :],
                                start=(kt == 0), stop=(kt == KT - 1))
                        nc.vector.tensor_copy(
                            qT_sb[:, dt, ib * BW:(ib + 1) * BW], ps)

                for jb in range(NB):
                    xb_blk = xblk.tile([P, KT, BW], f32r, tag="xblk")
                    nc.sync.dma_start(
                        out=xb_blk,
                        in_=xbT[:, jb * BW:(jb + 1) * BW]
                        .rearrange("(kt p) i -> p kt i", p=P))
                    for dt in range(HH):
                        ps = psum1.tile([P, BW], f32, tag="ps1")
                        for kt in range(KT):
                            nc.tensor.matmul(
                                ps,
                                wk_sb[:, kt, dt * P:(dt + 1) * P],
                                xb_blk[:, kt, :],
                                start=(kt == 0), stop=(kt == KT - 1))
                        nc.vector.tensor_copy(
                            kT_sb[:, dt, jb * BW:(jb + 1) * BW], ps)
                    for j2 in range(BW // P):
                        jt = jb * (BW // P) + j2
                        ps = psum1.tile([P, DVC], f32, tag="psv")
                        for kt in range(KT):
                            nc.tensor.matmul(
                                ps,
                                xb_blk[:, kt, j2 * P:(j2 + 1) * P],
                                wv_sb[:, kt, :],
                                start=(kt == 0), stop=(kt == KT - 1))
                        nc.vector.tensor_copy(v_sb[:, jt, :], ps)

            # ---------------- phase 2: attention ----------------
            with tc.tile_pool(name="expp", bufs=10) as expp, \
                 tc.tile_pool(name="bcp", bufs=2) as bcp, \
                 tc.tile_pool(name="rcp", bufs=2) as rcp, \
                 tc.tile_pool(name="dotsp", bufs=2, space="PSUM") as dotsp, \
                 tc.tile_pool(name="avp", bufs=1, space="PSUM") as avp, \
                 tc.tile_pool(name="denp", bufs=1, space="PSUM") as denp:
                for h in range(HH):
                    for ib in range(IB2):
                        i0 = ib * 1024
                        po = avp.tile([P, 1024], f32)
                        pd = denp.tile([1, 1024], f32)
                        for jt in range(NJT):
                            ps = dotsp.tile([P, 1024], f32)
                            k_l = kT_sb[:, h, jt * P:(jt + 1) * P]
                            for hf in range(2):
                                nc.tensor.matmul(
                                    ps[:, hf * 512:(hf + 1) * 512],
                                    k_l,
                                    qT_sb[:, h, i0 + hf * 512:i0 + (hf + 1) * 512]
                                    ,
                                    start=True, stop=True)
                            et = expp.tile([P, 1024], f32r, tag="exp")
                            nc.scalar.activation(et, ps, Exp, scale=SCALE)
                            v_l = v_sb[:, jt, h * P:(h + 1) * P]
                            for hf in range(2):
                                sl = slice(hf * 512, (hf + 1) * 512)
                                nc.tensor.matmul(
                                    po[:, sl], v_l, et[:, sl],
                                    start=(jt == 0), stop=(jt == NJT - 1))
                                nc.tensor.matmul(
                                    pd[:, sl], ones_sb,
                                    et[:, sl],
                                    start=(jt == 0), stop=(jt == NJT - 1))
                        # drain the PV accumulator to SBUF right away so the
                        # PSUM bank frees for the next block; normalize there.
                        osl = oT_sb[:, h, i0:i0 + 1024]
                        nc.vector.tensor_copy(osl, po)
                        rc = rcp.tile([1, 1024], f32, tag="rc")
                        nc.vector.reciprocal(rc, pd)
                        bc = bcp.tile([P, 1024], f32, tag="bc")
                        nc.gpsimd.partition_broadcast(bc, rc)
                        nc.vector.tensor_mul(osl, osl, bc)

            # ---------------- phase 3: output projection ----------------
            with tc.tile_pool(name="wop", bufs=1) as wop, \
                 tc.tile_pool(name="ystage", bufs=4) as ystage, \
                 tc.tile_pool(name="psum3", bufs=4, space="PSUM") as psum3:
                wo_sb = wop.tile([P, HH, DIM], f32r)
                nc.sync.dma_start(
                    out=wo_sb, in_=woT.rearrange("(dt p) e -> p dt e", p=P))
                for e8 in range(DIM // P):
                    for ib in range(NIB):
                        ps = psum3.tile([P, 512], f32)
                        for dt in range(HH):
                            nc.tensor.matmul(
                                ps,
                                wo_sb[:, dt, e8 * P:(e8 + 1) * P],
                                oT_sb[:, dt, ib * 512:(ib + 1) * 512]
                                ,
                                start=(dt == 0), stop=(dt == HH - 1))
                        ys = ystage.tile([P, 512], f32, tag="ys")
                        if ib % 2 == 0:
                            nc.vector.tensor_copy(ys, ps)
                        else:
                            nc.scalar.copy(ys, ps)
                        nc.sync.dma_start(
                            out=yT[e8 * P:(e8 + 1) * P, ib * 512:(ib + 1) * 512],
                            in_=ys)

    nc.compile()
    return nc


_nc_by_reps = {}


def _get_nc(reps=1):
    if reps not in _nc_by_reps:
        _nc_by_reps[reps] = _build_nc(reps)
    return _nc_by_reps[reps]


def kernel(x_a, x_b, W_q, W_kv, W_out, b_out):
    global LAST_EXEC_NS, LAST_RESULTS
    from concourse import bass_utils

    x_a = np.asarray(x_a, dtype=np.float32)
    x_b = np.asarray(x_b, dtype=np.float32)
    W_q = np.asarray(W_q, dtype=np.float32)
    W_kv = np.asarray(W_kv, dtype=np.float32)
    W_out = np.asarray(W_out, dtype=np.float32)
    b_out = np.asarray(b_out, dtype=np.float32)

    nc = _get_nc(REPS)

    xaT = [np.ascontiguousarray(x_a[b].T) for b in range(B)]
    xbT = [np.ascontiguousarray(x_b[b].T) for b in range(B)]
    in_maps = []
    for c in range(NCORES):
        b, hh = divmod(c, 2)
        hs = hh * DVC
        in_maps.append({
            "xaT": xaT[b],
            "xbT": xbT[b],
            "wqT": np.ascontiguousarray(W_q[hs:hs + DVC].T),
            "wkT": np.ascontiguousarray(W_kv[hs:hs + DVC].T),
            "wvT": np.ascontiguousarray(W_kv[DIM + hs:DIM + hs + DVC].T),
            "woT": np.ascontiguousarray(W_out[:, hs:hs + DVC].T),
            "ones": np.ones((P, 1), dtype=np.float32),
        })

    res = bass_utils.run_bass_kernel_spmd(
        nc, in_maps, core_ids=list(range(NCORES)), trace=_TRACE)
    LAST_EXEC_NS = res.exec_time_ns
    LAST_RESULTS = res

    out = np.empty((B, N, DIM), dtype=np.float32)
    for b in range(B):
        acc = res.results[2 * b]["yT"] + res.results[2 * b + 1]["yT"]
        out[b] = acc.T + b_out
    return out


def _make_in_maps(x_a, x_b, W_q, W_kv, W_out):
    xaT = [np.ascontiguousarray(x_a[b].T) for b in range(B)]
    xbT = [np.ascontiguousarray(x_b[b].T) for b in range(B)]
    in_maps = []
    for c in range(NCORES):
        b, hh = divmod(c, 2)
        hs = hh * DVC
        in_maps.append({
            "xaT": xaT[b],
            "xbT": xbT[b],
            "wqT": np.ascontiguousarray(W_q[hs:hs + DVC].T),
            "wkT": np.ascontiguousarray(W_kv[hs:hs + DVC].T),
            "wvT": np.ascontiguousarray(W_kv[DIM + hs:DIM + hs + DVC].T),
            "woT": np.ascontiguousarray(W_out[:, hs:hs + DVC].T),
            "ones": np.ones((P, 1), dtype=np.float32),
        })
    return in_maps


def bench(inputs, reps_pair=(1, 9), iters=5):
    """Measure on-device time per kernel body via rep-delta wall timing."""
    import time
    from concourse import bass_utils
    ins = {k: np.asarray(v, dtype=np.float32) for k, v in inputs.items()
           if k != "b_out"}
    in_maps = _make_in_maps(ins["x_a"], ins["x_b"], ins["W_q"], ins["W_kv"],
                            ins["W_out"])
    walls = {}
    for reps in reps_pair:
        nc = _get_nc(reps)
        # warm-up (compile+cache)
        bass_utils.run_bass_kernel_spmd(nc, in_maps, core_ids=list(range(NCORES)))
        ts = []
        for _ in range(iters):
            t0 = time.perf_counter()
            bass_utils.run_bass_kernel_spmd(nc, in_maps,
                                            core_ids=list(range(NCORES)))
            ts.append(time.perf_counter() - t0)
        walls[reps] = min(ts)
        print(f"reps={reps}: wall min={walls[reps]*1e3:.2f} ms  all={[f'{t*1e3:.1f}' for t in ts]}")
    r0, r1 = reps_pair
    ns = (walls[r1] - walls[r0]) / (r1 - r0) * 1e9
    print(f"per-body device time: {ns:.0f} ns")
    return ns
